# revision 32
# baseline (speedup 1.0000x reference)
"""Trainium2 Bass kernel for nn_Attention_85581518340337.

Restormer-style channel attention:
  x (1,64,16,64,64) -> 1x1x1 conv (64->768) -> grouped 3x3x3 conv (192 groups of 4)
  -> split q,k,v (4 heads x 64 ch) -> L2 normalize over n=t*h*w -> attn = softmax(q@kT * temp)
  -> out = attn@v -> 1x1x1 proj (256->64)

Sharding: spatial over H (64 rows -> 8 cores x 8 rows, halo 1 row each side).
Per core: folded (qkv1*dwconv) dense conv for q,k in fp8e4 DoubleRow (2 tap-slots
per matmul, K=256); x staged as three w-pre-shifted copies so every conv window
is a contiguous 512-run (2-free-dim APs). Conv output evacuated to rolling bf16
buffers, DMA-transposed per 1024-position group and reduced into per-pair Gram
matrices on PE, interleaved with the k-macro conv; per-pair 133KB AllReduce of
Gram partials (pair 0 launches at ~50% of the conv). Softmax + projection fold
into per-head 64x64 matrices; v never materializes: out = (B o Mfold_v) * x as
a bf16 conv with t-shifted slot pairs packed into M=128 matmuls.
"""

import numpy as np
import ml_dtypes

import concourse.bass as bass
import concourse.mybir as mybir
import concourse.tile as tile
from concourse import bacc
from concourse.bass_utils import run_bass_kernel_spmd

F32 = mybir.dt.float32
BF16 = mybir.dt.bfloat16
FP8 = mybir.dt.float8e4
DR = mybir.MatmulPerfMode.DoubleRow

N_CORES = 8
DIM = 64
HEADS = 4
T, H, W = 16, 64, 64
HL = H // N_CORES          # 8 output h-rows per core
HLH = HL + 2               # 10 h-rows incl halo
C3H = DIM * 3 * HEADS      # 768
N_LOC = T * HL * W         # 8192 output positions per core
NT = 512                   # one t-plane (8*64)
QK_SCALE = 128.0           # fp8 weight scale (cancels in normalization)

# staging layout: no w-padding; w-shifts live in pre-shifted copies
PT, PH = T + 2, HLH
PLANE2 = PH * W            # 640
FREE2 = PT * PLANE2        # 11520

# 14 tap-slots; all windows are [8 rows x 64 w] = contiguous 512 at their origin
#   s in 0..8  : buf0 (xa: band0=S(-1), band1=S(0)),  dti=s//3, dhi=s%3
#   s in 9..11 : buf1 (xb: band0=S(+1), band1=h+1 S(+1)), dti=s-9, dhi=0
#   s = 12     : buf2 (xc: band0=S(+1), band1=t+1 S(+1)), dti=0, dhi=2
#   s = 13     : buf1 band0 only, dti=2, dhi=2 (band1 weights zero)
def _worg(s, t):
    if s < 9:
        return (t + s // 3) * PLANE2 + (s % 3) * 64
    if s < 12:
        return FREE2 + (t + s - 9) * PLANE2
    if s == 12:
        return 2 * FREE2 + t * PLANE2 + 2 * 64
    return FREE2 + (t + 2) * PLANE2 + 2 * 64


# DoubleRow slot pairs for the q,k conv (dwt8 slot order = this, flattened)
DRP = [(0, 1), (2, 3), (4, 5), (6, 7), (8, 9), (10, 11), (13, 12)]
# v-conv uses only xa'/xb' (no xc'): 15 v-slots; the (.,2,2) taps become three
# band0-only singles on xb' (12,13,14), two of which t-pair with each other.
#   v-slot 0..8 : xa', dual-band, taps ((dti,dhi,0),(dti,dhi,1))
#   v-slot 9..11: xb', dual-band, taps ((dti,0,2),(dti,1,2))
#   v-slot 12+i : xb' band0 only, tap (i,2,2)
# t-shift pairs (A,B): window(B,t) == window(A,t+1) -> B parts of plane t land
# in plane t-1's psum rows 64:128. Then singles (M=64).
VPAIRS = [(0, 3), (1, 4), (2, 5), (9, 10), (12, 13)]
VSINGLES = [6, 7, 8, 11, 14]
VORD = [0, 3, 1, 4, 2, 5, 9, 10, 12, 13, 6, 7, 8, 11, 14]


def _worgv(s, t):
    if s < 9:
        return (t + s // 3) * PLANE2 + (s % 3) * 64
    if s < 12:
        return FREE2 + (t + s - 9) * PLANE2
    return FREE2 + (t + s - 12) * PLANE2 + 2 * 64

_CACHE = {}


def _declare_dram(nc):
    x8_d = nc.dram_tensor("x8", [3, DIM, FREE2], FP8, kind="ExternalInput").ap()
    xv_d = nc.dram_tensor("xv", [3, DIM, FREE2], BF16, kind="ExternalInput").ap()
    dwt_d = nc.dram_tensor("dwt", [4, 128, 14, 128], FP8, kind="ExternalInput").ap()
    dwtv_d = nc.dram_tensor("dwtv", [15, 2, 128, 128], BF16, kind="ExternalInput").ap()
    projt_d = nc.dram_tensor("projt", [128, 2, DIM], F32, kind="ExternalInput").ap()
    temp_d = nc.dram_tensor("temp", [HEADS], F32, kind="ExternalInput").ap()
    eye_d = nc.dram_tensor("eye", [128, 128], F32, kind="ExternalInput").ap()
    out_d = nc.dram_tensor("out", [DIM, T, HL, W], F32, kind="ExternalOutput").ap()
    return x8_d, xv_d, dwt_d, dwtv_d, projt_d, temp_d, eye_d, out_d


def _build(sim=False, stop_after=None):
    nc = bacc.Bacc("TRN2", target_bir_lowering=False, debug=False,
                   num_devices=1 if sim else N_CORES)
    tensors = _declare_dram(nc)
    with tile.TileContext(nc) as tc:
        _emit(nc, tc, *tensors, sim=sim, stop_after=stop_after)
    nc.compile()
    return nc


def _emit(nc, tc, x8_d, xv_d, dwt_d, dwtv_d, projt_d, temp_d, eye_d, out_d,
          sim=False, stop_after=None):
    import contextlib
    ctx = contextlib.ExitStack()
    with ctx:
        singles = ctx.enter_context(tc.tile_pool(name="singles", bufs=1))
        qkr_p = ctx.enter_context(tc.tile_pool(name="qkr", bufs=6))
        ct_p = ctx.enter_context(tc.tile_pool(name="ctp", bufs=1))
        small_p = ctx.enter_context(tc.tile_pool(name="small", bufs=2))
        out_p = ctx.enter_context(tc.tile_pool(name="outp", bufs=3))
        ps_conv = ctx.enter_context(tc.tile_pool(name="ps_conv", bufs=4, space="PSUM"))
        ps_gram = ctx.enter_context(tc.tile_pool(name="ps_gram", bufs=1, space="PSUM"))
        ps_fo = ctx.enter_context(tc.tile_pool(name="ps_fo", bufs=3, space="PSUM"))
        dram = ctx.enter_context(tc.tile_pool(name="dram", bufs=1, space="DRAM"))

        # ---- prefetch all conv weights (avoid PE stalls at macro starts) ----
        dwt_sbs = [singles.tile([128, 14, 128], FP8, name=f"dwt{m}")
                   for m in range(4)]
        for m in range(4):
            nc.sync.dma_start(out=dwt_sbs[m][:], in_=dwt_d[m])

        # ---- staged inputs: fp8 trio split across queues in 3 plane-chunks so
        # the conv can start after ~1/3 of the data; bf16 duo on the gpsimd
        # software queue (needed only by the late v-conv) ----
        x8 = singles.tile([128, 3 * FREE2], FP8)
        xv = singles.tile([128, 2 * FREE2], BF16)
        THIRD = 6 * PLANE2

        # bands: buf0=(S(-1),S(0)), buf1=(S(+1), h+1 S(+1)), buf2=(S(+1), t+1 S(+1))
        # one queue per buffer; first 6 planes of each buffer first so block 0
        # can start after ~1/3 of the staging
        for lo, hi in ((0, THIRD), (THIRD, FREE2)):
            nc.sync.dma_start(out=x8[0:64, lo:hi], in_=x8_d[0][:, lo:hi])
            nc.sync.dma_start(out=x8[64:128, lo:hi], in_=x8_d[1][:, lo:hi])
            nc.scalar.dma_start(out=x8[0:64, FREE2 + lo:FREE2 + hi],
                                in_=x8_d[2][:, lo:hi])
            b1hi = min(hi, FREE2 - 64)
            nc.scalar.dma_start(out=x8[64:128, FREE2 + lo:FREE2 + b1hi],
                                in_=x8_d[2][:, 64 + lo:64 + b1hi])
            nc.gpsimd.dma_start(out=x8[0:64, 2 * FREE2 + lo:2 * FREE2 + hi],
                                in_=x8_d[2][:, lo:hi])
            c1hi = min(hi, FREE2 - PLANE2)
            nc.gpsimd.dma_start(out=x8[64:128, 2 * FREE2 + lo:2 * FREE2 + c1hi],
                                in_=x8_d[2][:, PLANE2 + lo:PLANE2 + c1hi])
        nc.gpsimd.memset(x8[64:128, 2 * FREE2 - 64:2 * FREE2], 0.0)
        nc.gpsimd.memset(x8[64:128, 3 * FREE2 - PLANE2:3 * FREE2], 0.0)

        nc.gpsimd.dma_start(out=xv[0:64, 0:FREE2], in_=xv_d[0])
        nc.gpsimd.dma_start(out=xv[64:128, 0:FREE2], in_=xv_d[1])
        nc.gpsimd.dma_start(out=xv[0:64, FREE2:2 * FREE2], in_=xv_d[2])
        nc.gpsimd.dma_start(out=xv[64:128, FREE2:2 * FREE2 - 64],
                            in_=xv_d[2][:, 64:])
        nc.gpsimd.memset(xv[64:128, 2 * FREE2 - 64:2 * FREE2], 0.0)

        projt_sb = singles.tile([128, 2, DIM], F32)
        eye_sb = singles.tile([128, 128], F32)
        tsc = singles.tile([128, 2], F32)
        nc.sync.dma_start(out=projt_sb[:], in_=projt_d)
        nc.sync.dma_start(out=eye_sb[:], in_=eye_d)
        for p_ in range(2):
            for hf_ in range(2):
                src_ = bass.AP(tensor=temp_d.tensor, offset=2 * p_ + hf_,
                               ap=[[0, 64], [1, 1]])
                nc.sync.dma_start(out=tsc[hf_ * 64:(hf_ + 1) * 64, p_:p_ + 1], in_=src_)

        # transposed (pos, ch) storage for the live pair: [g, chunk, q|k]
        ct = None  # allocated per pair (tag reuse)
        gq_ps = [None, None]
        arbuf = singles.tile([128, 2, 130], F32)
        ssqk = singles.tile([128, 2, T], F32)

        x8f = x8[:]
        xvf = xv[:]

        def winv(s, t):
            o = _worgv(s, t)
            return bass.AP(tensor=xvf.tensor, offset=xvf.offset + o,
                           ap=[list(xvf.ap[0]), [1, NT]])

        def win_dr(j, t):
            s0, s1 = DRP[j]
            o0, o1 = _worg(s0, t), _worg(s1, t)
            return bass.AP(tensor=x8f.tensor, offset=x8f.offset + o0,
                           ap=[list(x8f.ap[0]), [o1 - o0, 2], [1, NT]])

        # ---- AllReduce plumbing ----
        ar_in = [dram.tile([128, 130], F32, name=f"ar_in{p}") for p in range(2)]
        ar_out = [dram.tile([128, 130], F32, name=f"ar_out{p}") for p in range(2)]
        gar = singles.tile([128, 2, 130], F32)

        def launch_ar(p):
            nc.gpsimd.dma_start(out=ar_in[p][:], in_=arbuf[:, p, :])
            if sim:
                nc.gpsimd.dma_start(out=ar_out[p][:], in_=ar_in[p][:])
            else:
                nc.gpsimd.collective_compute(
                    "AllReduce", mybir.AluOpType.add,
                    replica_groups=[list(range(N_CORES))],
                    ins=[ar_in[p].opt()], outs=[ar_out[p].opt()])
            nc.gpsimd.dma_start(out=gar[:, p, :], in_=ar_out[p][:])

        def gram_group(p, g):
            for j in range(8):
                jj = g * 8 + j
                nc.tensor.matmul(gq_ps[p][:], ct[:, g, j, 0:128], ct[:, g, j, :],
                                 start=(jj == 0), stop=(jj == 63))

        def extract_pair(p):
            nc.vector.tensor_copy(out=arbuf[:, p, 0:128], in_=gq_ps[p][:, 128:256])
            scr = small_p.tile([128, 128], F32, tag="scr")
            nc.vector.tensor_mul(scr[:], gq_ps[p][:, 0:128], eye_sb[:])
            nc.vector.tensor_reduce(out=arbuf[:, p, 128:129], in_=scr[:],
                                    axis=mybir.AxisListType.X, op=mybir.AluOpType.add)
            nc.vector.tensor_reduce(out=arbuf[:, p, 129:130], in_=ssqk[:, p, :],
                                    axis=mybir.AxisListType.X, op=mybir.AluOpType.add)

        def conv_macro(mac, pair, is_k, do_gram=True, pure=False):
            """fp8 DoubleRow folded conv for one 128-ch macro; slot-major over
            4-plane blocks. For k-macros, interleave per-group DMA transposes
            of (q,k) evac + Gram matmuls one block behind."""
            dwt_sb = dwt_sbs[mac]
            for tb in range(0, T, 2):
                g = tb // 2
                pss = [ps_conv.tile([128, NT], F32, tag="cps", name=f"cps{ti}")
                       for ti in range(2)]
                for ti in range(2):
                    for j in range(7):
                        nc.tensor.matmul(pss[ti][:], dwt_sb[:, 2 * j:2 * j + 2, :],
                                         win_dr(j, tb + ti),
                                         start=(j == 0), stop=(j == 6),
                                         perf_mode=DR)
                if pure:
                    continue
                qk_g = qkr_p.tile([128, 2, NT], BF16, tag="qkr", name=f"qkr{g}")
                for ti in range(2):
                    t = tb + ti
                    dst = qk_g[:, ti, :]
                    if ti == 0:
                        nc.vector.tensor_copy(out=dst, in_=pss[ti][:])
                    else:
                        nc.scalar.copy(out=dst, in_=pss[ti][:])
                    if is_k:
                        scr = small_p.tile([128, NT], F32, tag="ttr")
                        nc.vector.tensor_mul(scr[:], dst, dst)
                        nc.vector.tensor_reduce(out=ssqk[:, pair, t:t + 1],
                                                in_=scr[:],
                                                axis=mybir.AxisListType.X,
                                                op=mybir.AluOpType.add)
                col = slice(128, 256) if is_k else slice(0, 128)
                nc.scalar.dma_start(
                    out=ct[:, g, :, col],
                    in_=qk_g[:].rearrange("p a b -> p (a b)"),
                    transpose=True)
                if is_k and do_gram and g >= 2:
                    gram_group(pair, g - 2)
            if is_k and do_gram and not pure:
                gram_group(pair, 6)
                gram_group(pair, 7)
                extract_pair(pair)
                launch_ar(pair)

        if stop_after == "inputs":
            nc.gpsimd.dma_start(out=out_d[:, 0],
                                in_=x8[0:64, 0:NT].rearrange("p (h w) -> p h w", h=HL))
            return

        # ---- per-pair: normalization, softmax, B (emitted early for pair 0
        # so its DVE work overlaps the k1 conv) ----
        rno = singles.tile([128, 2, 2], F32)
        rqs = singles.tile([128, 2], F32)
        rk_d = [dram.tile([128, 1], F32, name=f"rk_d{p}") for p in range(2)]
        rkb = singles.tile([128, 2, 128], F32)
        bt_sb = [singles.tile([128, DIM], BF16, tag=f"bt{p}", name=f"bt{p}")
                 for p in range(2)]

        at_t = [None, None]

        def softmax_prep(p):
            """DVE/DMA-only part: normalized logits -> attn weights `at`.
            No PE instructions, so it can be emitted mid-conv."""
            nc.scalar.activation(out=rno[:, p, :], in_=gar[:, p, 128:130],
                                 func=mybir.ActivationFunctionType.Sqrt)
            nc.vector.reciprocal(out=rno[:, p, :], in_=rno[:, p, :])
            nc.vector.tensor_mul(rqs[:, p:p + 1], rno[:, p, 0:1], tsc[:, p:p + 1])
            nc.sync.dma_start(out=rk_d[p][:], in_=rno[:, p, 1:2])
            src = bass.AP(tensor=rk_d[p].tensor, offset=rk_d[p].offset,
                          ap=[[0, 128], [1, 128]])
            nc.sync.dma_start(out=rkb[:, p, :], in_=src)

            lg = small_p.tile([128, 128], F32, tag="lg")
            nc.vector.tensor_mul(lg[:], gar[:, p, 0:128], rkb[:, p, :])
            nc.vector.tensor_scalar_mul(lg[:], lg[:], rqs[:, p:p + 1])
            at = small_p.tile([128, 64], F32, tag=f"at{p}", name=f"at{p}")
            at_t[p] = at
            mx = small_p.tile([128, 1], F32, tag="mx")
            sm = small_p.tile([128, 1], F32, tag="sm")
            for hf in range(2):
                hs = slice(hf * 64, (hf + 1) * 64)
                sub = lg[hs, hf * 64:(hf + 1) * 64]
                nc.vector.tensor_reduce(out=mx[hs], in_=sub, axis=mybir.AxisListType.X,
                                        op=mybir.AluOpType.max, negate=True)
                nc.scalar.activation(out=at[hs], in_=sub,
                                     func=mybir.ActivationFunctionType.Exp,
                                     bias=mx[hs], scale=1.0)
                nc.vector.tensor_reduce(out=sm[hs], in_=at[hs], axis=mybir.AxisListType.X,
                                        op=mybir.AluOpType.add)
                nc.vector.reciprocal(out=sm[hs], in_=sm[hs])
                nc.vector.tensor_scalar_mul(at[hs], at[hs], sm[hs])

        def softmax_bt(p):
            """PE part: B^T = attn^T @ projT, emitted after the last conv."""
            at = at_t[p]
            btp = ps_conv.tile([128, DIM], F32, tag="cps", name=f"btp{p}")
            for hf in range(2):
                hs = slice(hf * 64, (hf + 1) * 64)
                nc.tensor.matmul(btp[hs, :], at[hs], projt_sb[hs, p, :],
                                 start=True, stop=True)
            nc.vector.tensor_copy(out=bt_sb[p][:], in_=btp[:])

        # ---- conv + gram + AR, pair by pair ----
        macro_order = [(0, 0, False), (2, 0, True), (1, 1, False), (3, 1, True)]
        for i, (mac, pair, is_k) in enumerate(macro_order):
            if not is_k and stop_after != "convpure":
                ct = ct_p.tile([128, 8, 8, 256], BF16, tag="ct", name=f"ct{pair}")
                gq_ps[pair] = ps_gram.tile([128, 256], F32, tag="gq",
                                           name=f"gq{pair}")
            conv_macro(mac, pair, is_k,
                       do_gram=(stop_after != "convonly"),
                       pure=(stop_after == "convpure"))
            if i == 2 and stop_after not in ("conv", "convonly", "convpure"):
                softmax_prep(0)

        if stop_after in ("conv", "convonly", "convpure"):
            return
        softmax_bt(0)
        softmax_prep(1)
        softmax_bt(1)

        if stop_after == "softmax":
            return

        # ---- compose G slots (VORD order so v-pairs are adjacent) ----
        dwtv_sb = singles.tile([128, 15, 2, 128], BF16)
        nc.gpsimd.dma_start(out=dwtv_sb[:], in_=dwtv_d.rearrange("s p k m -> k s p m"))
        gv = singles.tile([128, 15, DIM], BF16)
        for si in range(15):
            gts = ps_gram.tile([128, DIM], F32, tag="gq", name=f"gts{si}")
            nc.tensor.matmul(gts[:], dwtv_sb[:, si, 0, :], bt_sb[0][:],
                             start=True, stop=False)
            nc.tensor.matmul(gts[:], dwtv_sb[:, si, 1, :], bt_sb[1][:],
                             start=False, stop=True)
            nc.vector.tensor_copy(out=gv[:, si, :], in_=gts[:])

        # ---- v-conv: pairs write [A(t) | B(t-1)], singles write rows 0:64 ----
        ps_ring = {}
        for t in range(T + 1):
            ps_t = ps_fo.tile([128, NT], F32, tag="fo", name=f"fo{t}")
            ps_ring[t] = ps_t
            for vj in range(5):
                nc.tensor.matmul(ps_t[:], gv[:, 2 * vj:2 * vj + 2, :],
                                 winv(VPAIRS[vj][0], t),
                                 start=(vj == 0), stop=(t == T and vj == 4))
            if t < T:
                for si, s in enumerate(VSINGLES):
                    nc.tensor.matmul(ps_t[0:64, :], gv[:, 10 + si, :],
                                     winv(s, t),
                                     start=False, stop=(si == 4))
            if t >= 1:
                tp = t - 1
                ot = out_p.tile([64, NT], F32, tag="ot")
                if tp % 2 == 0:
                    nc.vector.tensor_copy(out=ot[:], in_=ps_ring[tp][0:64, :])
                else:
                    nc.scalar.copy(out=ot[:], in_=ps_ring[tp][0:64, :])
                nc.vector.tensor_add(ot[:], ot[:], ps_t[64:128, :])
                del ps_ring[tp]
                nc.sync.dma_start(out=out_d[:, tp],
                                  in_=ot[:].rearrange("p (h w) -> p h w", h=HL))


def _prep_inputs(x, qkv_w, dw_w, proj_w, temperature):
    """Host-side sharding + weight layout."""
    b, c, t, h, w = x.shape
    w1 = qkv_w.reshape(C3H, DIM).astype(np.float64)   # (768, 64)
    dw = dw_w.reshape(C3H, 4, 3, 3, 3).astype(np.float64)
    # folded conv: M[o, c, dti, dhi, dwi] = sum_j dw[o, j, taps] * w1[4*(o//4)+j, c]
    j_idx = (np.arange(C3H) // 4) * 4
    w1g = w1[j_idx[:, None] + np.arange(4)[None, :], :]      # (768, 4, 64)
    mfold = np.einsum("ojtuv,ojc->octuv", dw, w1g)           # (768, 64, 3,3,3)
    # slot -> (band0 tap, band1 tap)
    slots = []
    for dti in range(3):
        for dhi in range(3):
            slots.append(((dti, dhi, 0), (dti, dhi, 1)))     # A (xa)
    for dti in range(3):
        slots.append(((dti, 0, 2), (dti, 1, 2)))             # B (xb)
    slots.append(((0, 2, 2), (1, 2, 2)))                     # C (xc)
    slots.append(((2, 2, 2), None))                          # single (xb band0)

    qk_order = [s for pair in DRP for s in pair]
    dwt = np.zeros((4, 14, 128, 128), dtype=np.float32)
    for mac in range(4):
        osl = slice(mac * 128, (mac + 1) * 128)
        for si, sl in enumerate(qk_order):
            tap0, tap1 = slots[sl]
            dwt[mac, si, 0:64, :] = mfold[osl, :, tap0[0], tap0[1], tap0[2]].T
            if tap1 is not None:
                dwt[mac, si, 64:128, :] = mfold[osl, :, tap1[0], tap1[1], tap1[2]].T
    dwt = np.ascontiguousarray(dwt.transpose(0, 2, 1, 3)) * QK_SCALE
    dwt = dwt.astype(ml_dtypes.float8_e4m3)
    # dwtv[si, p, o, 64b + c] = mfold[512 + 128p + o, c, vtap(VORD[si], b)]
    vslots = slots[0:12] + [((0, 2, 2), None), ((1, 2, 2), None),
                            ((2, 2, 2), None)]
    dwtv = np.zeros((15, 2, 128, 128), dtype=np.float32)
    for si, sl in enumerate(VORD):
        tap0, tap1 = vslots[sl]
        for p in range(2):
            osl = slice(512 + p * 128, 512 + (p + 1) * 128)
            dwtv[si, p, :, 0:64] = mfold[osl, :, tap0[0], tap0[1], tap0[2]]
            if tap1 is not None:
                dwtv[si, p, :, 64:128] = mfold[osl, :, tap1[0], tap1[1], tap1[2]]
    dwtv = dwtv.astype(ml_dtypes.bfloat16)
    pw = proj_w.reshape(DIM, HEADS, DIM)              # (e, h, c)
    projt = np.zeros((128, 2, DIM), dtype=np.float32)
    for p in range(2):
        for hf in range(2):
            projt[hf * 64:(hf + 1) * 64, p, :] = pw[:, 2 * p + hf, :].T
    temp = np.asarray(temperature, dtype=np.float32).reshape(HEADS)
    eye = np.eye(128, dtype=np.float32)

    # padded slab per core: planes 1..16 = x, h-halo rows, then 3 w-shifts
    in_maps = []
    for i in range(N_CORES):
        xp = np.zeros((c, PT, PH, W), dtype=np.float32)
        h0 = i * HL - 1
        rlo, rhi = max(0, -h0), min(PH, h - h0)
        xp[:, 1:T + 1, rlo:rhi, :] = x[0][:, :, h0 + rlo:h0 + rhi, :]
        shifts = np.zeros((3, c, PT, PH, W), dtype=np.float32)
        shifts[0][:, :, :, 1:] = xp[:, :, :, :-1]    # S(-1): value x[w-1]
        shifts[1] = xp                               # S(0)
        shifts[2][:, :, :, :-1] = xp[:, :, :, 1:]    # S(+1): value x[w+1]
        sh = shifts.reshape(3, c, FREE2)
        in_maps.append({
            "x8": sh.astype(ml_dtypes.float8_e4m3),
            "xv": sh.astype(ml_dtypes.bfloat16),
            "dwt": dwt, "dwtv": dwtv, "projt": projt,
            "temp": temp, "eye": eye})
    return in_maps


def kernel(x, qkv_w, dw_w, proj_w, temperature, _trace=False):
    if "nc" not in _CACHE:
        _CACHE["nc"] = _build()
    nc = _CACHE["nc"]
    in_maps = _prep_inputs(np.asarray(x, np.float32), np.asarray(qkv_w, np.float32),
                           np.asarray(dw_w, np.float32), np.asarray(proj_w, np.float32),
                           np.asarray(temperature, np.float32))
    kw = {}
    if _trace:
        kw = dict(trace=True, stitch_traces=True, trace_cores=list(range(N_CORES)))
    res = run_bass_kernel_spmd(nc, in_maps, core_ids=list(range(N_CORES)), **kw)
    _CACHE["last_res"] = res
    out = np.zeros((1, DIM, T, H, W), dtype=np.float32)
    for i in range(N_CORES):
        out[0, :, :, i * HL:(i + 1) * HL, :] = res.results[i]["out"]
    return out
